# revision 35
# baseline (speedup 1.0000x reference)
"""GCN layer kernel for Trainium2, 8-core SPMD.

Computes: out = (A @ (X @ W + b)) / colsum(A)[:, None],  A = (adj != 0)
with N=8192 nodes, F_in=F_out=512, across 8 NeuronCores.

Sharding: row-shard adjacency and node features (1024 rows per core),
replicate W/b. Degree (column sums of A) needs rows from every core: each
core computes partial column sums for free via the binarize op's
accumulate output, an all-gather shares them, and an on-device tree sum
+ reciprocal finishes the normalization.

Hidden-projection strategy: the rank-entry barrier plus the H AllGather
stall the main matmuls for ~80-100us at kernel start. Instead of idling
(the HAM clock gate would drop the PE to 1.2 GHz), each core computes H
= X@W+b for ITSELF AND THE NEXT LOCAL_RANKS-1 ranks from cheap extra X
inputs. The main loop visits contraction tiles in per-core rotated order
kt=(pid*8+i)%64, so its first LOCAL_RANKS*8 iterations use locally
computed H blocks and never wait on the AllGather; only the far half
reads gathered data. The remaining pre-gather slack is bridged with a
few discarded f32 warm-up matmuls.

Other measured-on-hw notes:
- Collective activity throttles regular DMA, so the A stream prefetches
  deeply (bf16 a_bin tiles).
- All heavy matmuls are bf16: A entries are exactly 0/1 so the lhsT is
  exact; only H carries bf16 rounding (~3e-3 final rel err).
- b is added via a K=1 matmul into PSUM, so the PSUM->SBUF evacuation is
  a plain copy on the otherwise idle Scalar engine.
"""
import numpy as np

N = 8192
F = 512
N_CORES = 8
NB = N // N_CORES          # 1024 rows per core
KT = N // 128              # 64 contraction tiles
MT = NB // 128             # 8 output row tiles per core
FI_T = F // 128            # 4 feat-in tiles
LR = 2                     # ranks whose H we compute locally
N_DUMMY = 70               # junk warm-up matmuls (~1.1us each, f32 4-pass)

_cached = {}


def _build():
    import concourse.bacc as bacc
    import concourse.bass as bass
    import concourse.tile as tile
    from concourse import mybir

    f32 = mybir.dt.float32
    bf16 = mybir.dt.bfloat16

    nc = bacc.Bacc("TRN2", target_bir_lowering=False, debug=False,
                   num_devices=N_CORES)
    at = nc.dram_tensor("at", [N, NB], f32, kind="ExternalInput").ap()
    xt = nc.dram_tensor("xt", [F, LR * NB], f32, kind="ExternalInput").ap()
    w = nc.dram_tensor("w", [F, F], f32, kind="ExternalInput").ap()
    bfull = nc.dram_tensor("bfull", [128, F], f32, kind="ExternalInput").ap()
    out = nc.dram_tensor("out", [NB, F], f32, kind="ExternalOutput").ap()

    pid = nc.partition_id()

    with tile.TileContext(nc) as tc:
        with tc.tile_pool(name="dram", bufs=1, space="DRAM") as dram, \
             tc.tile_pool(name="p", bufs=1) as p, \
             tc.tile_pool(name="ps", bufs=1, space="PSUM") as ps:
            hg_in = dram.tile([NB, F], bf16)
            hg_out = dram.tile([N, F], bf16, addr_space="Shared")
            dg_in = dram.tile([128, KT], f32)
            dg_out = dram.tile([128 * N_CORES, KT], f32, addr_space="Shared")

            cs = p.tile([128, KT], f32)    # per-core partial column sums
            ones = p.tile([128, NB], f32)
            nc.vector.memset(ones[:], 1.0)
            ones1 = p.tile([1, 128], bf16)
            nc.vector.memset(ones1[:], 1.0)

            # ---- W and b staged to bf16 ----
            w_c = p.tile([128, FI_T * F], bf16)
            for ki in range(FI_T):
                stg_w = p.tile([128, F], f32, tag="stgw", bufs=2,
                               name=f"stgw{ki}")
                nc.scalar.dma_start(stg_w[:], w[ki * 128:(ki + 1) * 128, :])
                nc.vector.tensor_copy(w_c[:, ki * F:(ki + 1) * F], stg_w[:])
            b_sb = p.tile([128, F], f32)
            nc.scalar.dma_start(b_sb[:], bfull)
            b_bf = p.tile([1, F], bf16)
            nc.vector.tensor_copy(b_bf[:], b_sb[0:1, :])

            # single PSUM pool: 8 banks, all held by the main accumulators;
            # H compute and warm-up reuse them as scratch (the first real
            # matmul's start=True clears each bank).
            pms = []
            for m in range(MT):
                pm = ps.tile([128, F], f32, tag=f"pm{m}", name=f"pm{m}",
                             bufs=1)
                pms.append(pm)

            # ---- H blocks for ranks pid..pid+LR-1 (bf16 matmuls) ----
            # bias lands via a K=1 matmul; evacuation is an ACT copy.
            hb_all = []
            for rr in range(LR):
                chunks = []
                for ki in range(FI_T):
                    stg_x = p.tile([128, NB], f32, tag="stgx", bufs=3,
                                   name=f"stgx{rr}_{ki}")
                    nc.scalar.dma_start(
                        stg_x[:],
                        xt[ki * 128:(ki + 1) * 128, rr * NB:(rr + 1) * NB])
                    xtc = p.tile([128, NB], bf16, tag="xtc", bufs=8,
                                 name=f"xtc{rr}_{ki}")
                    nc.vector.tensor_copy(xtc[:], stg_x[:])
                    chunks.append(xtc)
                for nt in range(MT):
                    hp = pms[nt % 2]
                    for ki in range(FI_T):
                        nc.tensor.matmul(
                            hp[:],
                            chunks[ki][:, nt * 128:(nt + 1) * 128],
                            w_c[:, ki * F:(ki + 1) * F],
                            start=(ki == 0), stop=False)
                    nc.tensor.matmul(hp[:], ones1[:], b_bf[:],
                                     start=False, stop=True)
                    hb = p.tile([128, F], bf16, tag="hb", bufs=LR * MT,
                                name=f"hb{rr}_{nt}")
                    nc.scalar.copy(hb[:], hp[:])
                    if rr == 0:
                        nc.gpsimd.dma_start(
                            hg_in[nt * 128:(nt + 1) * 128, :], hb[:])
                    hb_all.append(hb)

            # ---- all-gather projected hidden ----
            nc.gpsimd.collective_compute(
                "AllGather", mybir.AluOpType.bypass,
                replica_groups=[list(range(N_CORES))],
                ins=[hg_in.opt()], outs=[hg_out.opt()],
            )

            # PE warm-up: a few slow f32 (4-pass) junk matmuls bridge any
            # remaining pre-gather slack; results are cleared by the first
            # real matmul's start=True.
            for j in range(N_DUMMY):
                nc.tensor.matmul(pms[j % MT][:], ones[:, 0:128],
                                 ones[:, 0:F], start=True, stop=True)

            # Main loop, rotated per core: iteration i handles physical tile
            # kt = (pid*8 + i) mod 64. The first LR*8 iterations use the
            # locally computed H blocks (no AllGather dependency); the rest
            # read the gathered hidden. PSUM accumulation is commutative.
            a_raws = []
            for i in range(KT):
                kt_e = (pid * MT + i) % KT
                a_raw = p.tile([128, NB], f32, tag="araw", bufs=8,
                               name=f"araw{i}")
                nc.sync.dma_start(a_raw[:], at[bass.ds(kt_e * 128, 128), :])
                a_raws.append(a_raw)

            for i in range(KT):
                kt_e = (pid * MT + i) % KT
                a_bin = p.tile([128, NB], bf16, tag="abin", bufs=36,
                               name=f"abin{i}")
                # one DVE op: a_bin = (a_raw != 0) * 1.0 (bf16, exact),
                # accum_out = free-dim sums = partial column sums of A.
                # cs is in ITERATION order (static AP keeps the dynamic-
                # offset register setup off the DVE hot path); the degree
                # combine below rotates per-rank slices to compensate.
                nc.vector.scalar_tensor_tensor(
                    a_bin[:], a_raws[i][:], 0.0, ones[:],
                    mybir.AluOpType.not_equal, mybir.AluOpType.mult,
                    accum_out=cs[:, i:i + 1])
                if i < LR * MT:
                    rhs = hb_all[i][:]
                else:
                    h_t = p.tile([128, F], bf16, tag="ht", bufs=16,
                                 name=f"ht{i}")
                    nc.scalar.dma_start(h_t[:],
                                        hg_out[bass.ds(kt_e * 128, 128), :])
                    rhs = h_t[:]
                for m in range(MT):
                    nc.tensor.matmul(
                        pms[m][:],
                        a_bin[:, m * 128:(m + 1) * 128],
                        rhs,
                        start=(i == 0), stop=(i == KT - 1))

            # ---- phase 3: degree + normalize ----
            nc.sync.dma_start(dg_in[:], cs[:])
            nc.gpsimd.collective_compute(
                "AllGather", mybir.AluOpType.bypass,
                replica_groups=[list(range(N_CORES))],
                ins=[dg_in.opt()], outs=[dg_out.opt()],
            )
            # pull each rank's partial for OUR column block: rank r stores
            # kt=(r*8+i)%64 at iteration-column i, so our block (kt=pid*8+m)
            # sits at columns [((pid-r)%8)*8, +8) of rank r's slab
            deg = p.tile([128, MT], f32)
            prt0 = p.tile([128, MT], f32, tag="prt", bufs=4, name="prt0")
            nc.gpsimd.dma_start(prt0[:], dg_out[0:128, bass.ts(pid, MT)])
            nc.vector.tensor_copy(deg[:], prt0[:])
            for r in range(1, N_CORES):
                col = ((pid + (N_CORES - r)) % N_CORES) * MT
                prt = p.tile([128, MT], f32, tag="prt", bufs=4,
                             name=f"prt{r}")
                nc.gpsimd.dma_start(
                    prt[:],
                    dg_out[r * 128:(r + 1) * 128, bass.ds(col, MT)])
                nc.vector.tensor_tensor(deg[:], deg[:], prt[:],
                                        mybir.AluOpType.add)
            rdeg = p.tile([128, MT], f32)
            nc.vector.reciprocal(rdeg[:], deg[:])

            for m in range(MT):
                o_sb = p.tile([128, F], f32, tag="osb", bufs=4,
                              name=f"osb{m}")
                if m % 2 == 0:
                    nc.vector.tensor_scalar(o_sb[:], pms[m][:],
                                            rdeg[:, m:m + 1], None,
                                            mybir.AluOpType.mult)
                else:
                    nc.scalar.mul(o_sb[:], pms[m][:], rdeg[:, m:m + 1])
                nc.sync.dma_start(out[m * 128:(m + 1) * 128, :], o_sb[:])

    nc.compile()
    return nc


def _get_nc():
    if "nc" not in _cached:
        _cached["nc"] = _build()
    return _cached["nc"]


def kernel(input_features, adj, W, b):
    from concourse.bass_utils import run_bass_kernel_spmd

    x = np.ascontiguousarray(np.asarray(input_features, dtype=np.float32))
    a = np.asarray(adj, dtype=np.float32)
    wm = np.ascontiguousarray(np.asarray(W, dtype=np.float32))
    bv = np.asarray(b, dtype=np.float32)
    bfull = np.ascontiguousarray(np.broadcast_to(bv, (128, F)))

    xts = [np.ascontiguousarray(x[k * NB:(k + 1) * NB, :].T)
           for k in range(N_CORES)]

    nc = _get_nc()
    in_maps = []
    for k in range(N_CORES):
        blk = slice(k * NB, (k + 1) * NB)
        xt_cat = np.concatenate(
            [xts[(k + rr) % N_CORES] for rr in range(LR)], axis=1)
        in_maps.append({
            "at": np.ascontiguousarray(a[blk, :].T),
            "xt": np.ascontiguousarray(xt_cat),
            "w": wm,
            "bfull": bfull,
        })
    res = run_bass_kernel_spmd(nc, in_maps, core_ids=list(range(N_CORES)))
    return np.concatenate([res.results[k]["out"] for k in range(N_CORES)],
                          axis=0)
